# revision 10
# baseline (speedup 1.0000x reference)
"""Multi-head attention (b=2, n=2048, d_model=1024, h=16, d_k=d_v=64) + relu(fc) +
residual + LayerNorm, sharded over 8 NeuronCores.

Sharding: core i = (batch bi = i//4) x (head-group hg = i%4, 4 heads each).

v2 design (exp-paced pipeline):
- The scalar-engine exp of the 4 heads x 2048 x 2048 scores (~17M elements at
  ~1 elem/lane/cycle) is the hard floor (~140us); every other engine is
  scheduled to stream underneath it.  Tensor-engine execution order ==
  emission order, so the kernel emits, per score group: scores(g) [bf16, two
  heads row-paired], exp(g) [fp8 out], ctx(g-1) [fp8 DoubleRow over the chunk
  pair], plus "filler" matmuls (projections / fc) that are never gated on
  recent results.  This keeps the PE warm (no >3.4us idle, no HAM
  re-throttle) and the scalar engine saturated.
- fp8e4 DoubleRow halves projection/ctx/fc matmul stream time (contraction
  256 per pass).  The attention path contributes only ~1% of the output
  magnitude (residual + LN dominate), so fp8 there is numerically safe.  wv
  and wfc are pre-scaled x16 so fp8 ctx values avoid subnormals; the x1/256
  is folded into the relu's tensor_scalar.
- A ones column rides in the v weights so the softmax denominator lands in
  psum row 64 of the ctx matmul; reciprocal via the fast-approx DVE op and a
  DRAM round-trip broadcast.
- fc partials ReduceScatter (4 ranks) per 512-query slab, split in two
  256-row chunks; relu+residual+LN run per-slab one block after the RS was
  issued so no engine FIFO head-blocks on the collective.
"""

import numpy as np
import ml_dtypes
from contextlib import ExitStack

B = 2
N = 2048
D = 1024
H = 16
DK = 64
HL = H // 4          # heads per core
CSL = HL * DK        # 256 per-core fc contraction
ROWS = N // 4        # 512 output rows per core
VW = 80              # padded ctx weight cols (64 v + 1 ones + 15 pad)
LN_EPS = 1e-6
N_CORES = 8
CTX_FP8 = True       # fp8 DoubleRow ctx path (False: bf16 per-chunk ctx)
RECIP_APPROX = False  # approx reciprocal corrupts PSUM-sourced [1,512] rows

_CACHE = {}


def _build():
    import concourse.bass as bass
    import concourse.tile as tile
    import concourse.mybir as mybir
    from concourse import bacc

    bf16 = mybir.dt.bfloat16
    fp8 = mybir.dt.float8e4
    f32 = mybir.dt.float32
    AF = mybir.ActivationFunctionType
    Alu = mybir.AluOpType
    DR = mybir.MatmulPerfMode.DoubleRow

    nc = bacc.Bacc("TRN2", target_bir_lowering=False, debug=False,
                   num_devices=N_CORES)

    qT = nc.dram_tensor("qT", [D, N], fp8, kind="ExternalInput").ap()
    kT = nc.dram_tensor("kT", [D, N], fp8, kind="ExternalInput").ap()
    vT = nc.dram_tensor("vT", [D, N], fp8, kind="ExternalInput").ap()
    wq = nc.dram_tensor("wq", [D, CSL], fp8, kind="ExternalInput").ap()
    wk = nc.dram_tensor("wk", [D, CSL], fp8, kind="ExternalInput").ap()
    wv = nc.dram_tensor("wv", [D, CSL], fp8, kind="ExternalInput").ap()
    wfc = nc.dram_tensor("wfc", [CSL, D], fp8, kind="ExternalInput").ap()
    qres = nc.dram_tensor("qres", [ROWS, D], f32, kind="ExternalInput").ap()
    gamma = nc.dram_tensor("gamma", [D], f32, kind="ExternalInput").ap()
    beta = nc.dram_tensor("beta", [D], f32, kind="ExternalInput").ap()
    y = nc.dram_tensor("y", [ROWS, D], f32, kind="ExternalOutput").ap()

    KC = D // 128     # 8 contraction chunks for projections
    KP = KC // 2      # 4 DoubleRow chunk pairs
    ST = N // 512     # 4 seq tiles of 512 queries
    SC = N // 128     # 16 seq chunks of 128 keys
    G = 2             # key chunks per group (exp batch == DoubleRow pair)
    NG = SC // G

    with tile.TileContext(nc) as tc:
        with ExitStack() as ctx:
            persist = ctx.enter_context(tc.tile_pool(name="persist", bufs=1))
            work = ctx.enter_context(tc.tile_pool(name="work", bufs=2))
            epool = ctx.enter_context(tc.tile_pool(name="epool", bufs=4))
            pat = ctx.enter_context(tc.tile_pool(name="pat", bufs=1, space="PSUM"))
            dram = ctx.enter_context(tc.tile_pool(name="dram", bufs=2, space="DRAM"))
            qkv_ctx = ExitStack()
            qkv = qkv_ctx.enter_context(tc.tile_pool(name="qkv", bufs=1))

            # PSUM: "s" score tiles [128,2,512] (2 banks) x3 = 6 banks;
            # "c" ctx tiles [80,512] (1 bank) x2.  Projection/fc psums borrow
            # "s" slots.
            def ps_s():
                return pat.tile([128, G, 512], f32, tag="s", name="ps_s", bufs=3)

            def ps_c():
                return pat.tile([VW, 512], f32, tag="c", name="ps_c", bufs=2)

            def ps_f(n=512):
                return pat.tile([128, n], f32, tag="s", name="ps_f", bufs=3)

            # ---- input tiles -------------------------------------------------
            qT_sb = qkv.tile([128, KC, N], fp8, tag="qT", name="qT")
            kT_sb = qkv.tile([128, KC, N], fp8, tag="kT", name="kT")
            vT_sb = qkv.tile([128, KC, N], fp8, tag="vT", name="vT")
            wq_sb = qkv.tile([128, KC, CSL], fp8, tag="wq", name="wq")
            wk_sb = qkv.tile([128, KC, CSL], fp8, tag="wk", name="wk")
            wv_sb = qkv.tile([128, KC, CSL], fp8, tag="wv", name="wv")
            wfc_sb = persist.tile([128, CSL // 128, D], fp8, tag="wfc", name="wfc")
            qres_sb = persist.tile([128, ST, D], f32, tag="qres", name="qres")
            gamma_sb = persist.tile([128, D], f32, tag="gamma", name="gamma")
            beta_sb = persist.tile([128, D], f32, tag="beta", name="beta")
            eps_sb = persist.tile([128, 1], f32, tag="eps", name="eps")

            # DMA issue order == arrival order: weights first, then kT
            # (k-proj needs the full sequence), qT slab 0, vT slab 0, then the
            # rest slab-interleaved.
            nc.sync.dma_start(out=wq_sb, in_=wq.rearrange("(c p) m -> p c m", p=128))
            nc.sync.dma_start(out=wk_sb, in_=wk.rearrange("(c p) m -> p c m", p=128))
            nc.sync.dma_start(out=wv_sb, in_=wv.rearrange("(c p) m -> p c m", p=128))
            nc.sync.dma_start(out=wfc_sb, in_=wfc.rearrange("(c p) n -> p c n", p=128))
            for st in range(ST):
                for kc in range(KC):
                    nc.sync.dma_start(
                        out=kT_sb[:, kc, st * 512:(st + 1) * 512],
                        in_=kT[kc * 128:(kc + 1) * 128, st * 512:(st + 1) * 512])
            for kc in range(KC):
                nc.sync.dma_start(out=qT_sb[:, kc, 0:512],
                                  in_=qT[kc * 128:(kc + 1) * 128, 0:512])
            for kc in range(KC):
                nc.sync.dma_start(out=vT_sb[:, kc, 0:512],
                                  in_=vT[kc * 128:(kc + 1) * 128, 0:512])
            for st in range(1, ST):
                for kc in range(KC):
                    nc.sync.dma_start(
                        out=vT_sb[:, kc, st * 512:(st + 1) * 512],
                        in_=vT[kc * 128:(kc + 1) * 128, st * 512:(st + 1) * 512])
                for kc in range(KC):
                    nc.sync.dma_start(
                        out=qT_sb[:, kc, st * 512:(st + 1) * 512],
                        in_=qT[kc * 128:(kc + 1) * 128, st * 512:(st + 1) * 512])
            nc.sync.dma_start(out=qres_sb, in_=qres.rearrange("(c p) n -> p c n", p=128))
            nc.sync.dma_start(out=gamma_sb,
                              in_=bass.AP(tensor=gamma.tensor, offset=gamma.offset,
                                          ap=[[0, 128]] + gamma.ap))
            nc.sync.dma_start(out=beta_sb,
                              in_=bass.AP(tensor=beta.tensor, offset=beta.offset,
                                          ap=[[0, 128]] + beta.ap))
            nc.vector.memset(eps_sb, LN_EPS)

            # ---- persistent activation tiles --------------------------------
            qhT = [persist.tile([128, N], bf16, tag=f"qhT{p}", name=f"qhT{p}") for p in range(2)]
            khT = [persist.tile([128, N], bf16, tag=f"khT{p}", name=f"khT{p}") for p in range(2)]
            # vh[g]: fp8 DoubleRow ctx weights, [keys 128, pair 2, head 4, VW]
            # cols 0-63 = 16*v, col 64 = ones (denominator), 65-79 zero pad.
            vh = [persist.tile([128, G, HL, VW], fp8 if CTX_FP8 else bf16,
                               tag=f"vh{g}", name=f"vh{g}")
                  for g in range(NG)]
            # normalized ctx (x16), fp8, [c 128 (2 heads), cc 2, q N]
            ctxn = persist.tile([128, 2, N], fp8, tag="ctxn", name="ctxn")
            xacc = qres_sb  # relu+residual accumulates in place over the residual

            for g in range(NG):
                nc.vector.memset(vh[g][:, :, :, DK:], 0.0)
                nc.vector.memset(vh[g][:, :, :, DK:DK + 1], 1.0)

            # ---- PE warm-up: dummy matmuls during the initial DMA -----------
            warm = persist.tile([128, 384], bf16, tag="warm", name="warm")
            nc.vector.memset(warm, 0.0)
            for i in range(16):
                ps = ps_f(256)
                nc.tensor.matmul(ps, warm[:, 0:128], warm[:, 0:256],
                                 start=True, stop=True)

            # ---- projections (fp8 DoubleRow, contraction pairs over kc) -----
            def proj(dst, p, st, w_sb, src):
                ps = ps_f()
                for kp in range(KP):
                    nc.tensor.matmul(
                        ps,
                        w_sb[:, 2 * kp:2 * kp + 2, p * 128:(p + 1) * 128],
                        src[:, 2 * kp:2 * kp + 2, st * 512:(st + 1) * 512],
                        start=(kp == 0), stop=(kp == KP - 1), perf_mode=DR)
                nc.vector.tensor_copy(out=dst[p][:, st * 512:(st + 1) * 512], in_=ps)

            def k_proj(p, st):
                proj(khT, p, st, wk_sb, kT_sb)

            def q_proj(p, st):
                proj(qhT, p, st, wq_sb, qT_sb)

            def v_proj(sc):
                # out: [seq 128, h*dk 256] = vT_chunk.T @ (16*wv); lands in the
                # DoubleRow weight tile for group sc//2, pair sc%2.
                ps = ps_f(CSL)
                for kp in range(KP):
                    nc.tensor.matmul(
                        ps,
                        vT_sb[:, 2 * kp:2 * kp + 2, sc * 128:(sc + 1) * 128],
                        wv_sb[:, 2 * kp:2 * kp + 2, :],
                        start=(kp == 0), stop=(kp == KP - 1), perf_mode=DR)
                nc.vector.tensor_copy(
                    out=vh[sc // G][:, sc % G, :, 0:DK],
                    in_=ps.rearrange("p (h d) -> p h d", h=HL))

            # ---- attention block (p, t): exp-paced emission -----------------
            def attention(p, t, extra=None):
                pc = [ps_c() for _ in range(2)]
                ppss = {}
                pse = {}
                for g in range(NG):
                    for s in range(2):
                        lo, hi = 64 * s, 64 * (s + 1)
                        ppss[s] = ps_s()
                        for j in range(G):
                            kc = g * G + j
                            nc.tensor.matmul(
                                ppss[s][:, j, :],
                                khT[p][lo:hi, kc * 128:(kc + 1) * 128],
                                qhT[p][lo:hi, t * 512:(t + 1) * 512],
                                start=True, stop=True)
                    for s in range(2):
                        pse[(g, s)] = epool.tile([128, G, 512],
                                                 fp8 if CTX_FP8 else bf16,
                                                 tag="e", name="e")
                        nc.scalar.activation(out=pse[(g, s)], in_=ppss[s], func=AF.Exp,
                                             scale=1.0 / float(np.sqrt(DK)))

                    def ctx_mm(gg, s, stop):
                        if CTX_FP8:
                            nc.tensor.matmul(
                                pc[s], vh[gg][:, :, 2 * p + s, :], pse[(gg, s)],
                                start=(gg == 0), stop=stop, perf_mode=DR)
                        else:
                            for j in range(G):
                                nc.tensor.matmul(
                                    pc[s][0:DK + 1, :],
                                    vh[gg][:, j, 2 * p + s, 0:DK + 1],
                                    pse[(gg, s)][:, j, :],
                                    start=(gg == 0 and j == 0),
                                    stop=(stop and j == G - 1))

                    if g > 0:
                        for s in range(2):
                            ctx_mm(g - 1, s, False)
                    if extra is not None:
                        extra(g)
                for s in range(2):
                    ctx_mm(NG - 1, s, True)
                # normalization: rb = 1/denominator broadcast via DRAM round
                # trip; ctxn = ctx16 * rb (fp8 out).
                rbs = []
                for s in range(2):
                    rb1 = work.tile([1, 512], f32, tag="rb1", name="rb1")
                    if RECIP_APPROX:
                        nc.vector.reciprocal_approx_fast(out=rb1, in_=pc[s][DK:DK + 1, :])
                    else:
                        nc.vector.reciprocal(out=rb1, in_=pc[s][DK:DK + 1, :])
                    r_dram = dram.tile([1, 512], f32, tag="rd", name="rd")
                    nc.gpsimd.dma_start(out=r_dram, in_=rb1)
                    rb = work.tile([DK, 512], f32, tag="rb", name="rb")
                    nc.gpsimd.dma_start(
                        out=rb,
                        in_=bass.AP(tensor=r_dram.tensor, offset=r_dram.offset,
                                    ap=[[0, DK]] + r_dram.ap[1:]))
                    rbs.append(rb)
                for s in range(2):
                    cun = work.tile([DK, 512], f32, tag="cun", name="cun")
                    nc.vector.tensor_copy(out=cun, in_=pc[s][0:DK, :])
                    nc.vector.tensor_mul(
                        out=ctxn[64 * s:64 * (s + 1), p, t * 512:(t + 1) * 512],
                        in0=cun, in1=rbs[s])

            # ---- fc + ReduceScatter per slab --------------------------------
            rs_bufs = {}

            def fc_tile(t, qq, nh):
                rs_in = rs_bufs[t][0]
                qc = t * 4 + qq
                ps = ps_f()
                nc.tensor.matmul(
                    ps,
                    ctxn[:, :, qc * 128:(qc + 1) * 128],
                    wfc_sb[:, :, nh * 512:(nh + 1) * 512],
                    start=True, stop=True, perf_mode=DR)
                fcs = work.tile([128, 512], bf16, tag="fcs", name="fcs")
                nc.vector.tensor_copy(out=fcs, in_=ps)
                nc.sync.dma_start(
                    out=rs_in[qq * 128:(qq + 1) * 128, nh * 512:(nh + 1) * 512],
                    in_=fcs)

            def rs_issue(t, half):
                # ReduceScatter rows [t*512 + half*256, +256) over 4 ranks;
                # each rank keeps 64 contiguous rows.
                rs_in = rs_bufs[t][0]
                rs_out = dram.tile([64, D], bf16, tag=f"rs_out{half}", name="rs_out")
                rs_bufs[t][1].append(rs_out)
                nc.gpsimd.collective_compute(
                    "ReduceScatter",
                    mybir.AluOpType.add,
                    replica_groups=[[0, 1, 2, 3], [4, 5, 6, 7]],
                    ins=[rs_in[half * 256:(half + 1) * 256, :].opt()],
                    outs=[rs_out.opt()])

            def fc_rs_units(t):
                rs_in = dram.tile([512, D], bf16, tag="rs_in", name="rs_in")
                rs_bufs[t] = (rs_in, [])
                units = []
                for qq in range(4):
                    for nh in range(2):
                        units.append(lambda t=t, qq=qq, nh=nh: fc_tile(t, qq, nh))
                    if qq == 1:
                        units.append(lambda t=t: rs_issue(t, 0))
                    if qq == 3:
                        units.append(lambda t=t: rs_issue(t, 1))
                return units

            def post_rs_a(t):
                # gather the RS result, relu(sum/256) + residual, LN stats
                rs_sb = work.tile([128, D], bf16, tag="rs_sb", name="rs_sb")
                for half, rs_out in enumerate(rs_bufs[t][1]):
                    nc.sync.dma_start(out=rs_sb[64 * half:64 * half + 64, :],
                                      in_=rs_out)
                xr = work.tile([128, D], f32, tag="xr", name="xr")
                nc.vector.tensor_scalar(out=xr, in0=rs_sb,
                                        scalar1=1.0 / 256.0, scalar2=0.0,
                                        op0=Alu.mult, op1=Alu.max)
                nc.vector.tensor_add(out=xacc[:, t, :], in0=xr,
                                     in1=qres_sb[:, t, :])
                x = xacc[:, t, :]
                stats = work.tile([128, 2, 6], f32, tag="stats", name="stats")
                nc.vector.bn_stats(out=stats[:, 0, :], in_=x[:, 0:512])
                nc.vector.bn_stats(out=stats[:, 1, :], in_=x[:, 512:1024])
                mv = work.tile([128, 2], f32, tag="mv", name="mv")
                nc.vector.bn_aggr(out=mv, in_=stats)
                rs_bufs[t] = (None, rs_bufs[t][1], mv)

            def post_rs_b(t):
                mv = rs_bufs[t][2]
                x = xacc[:, t, :]
                nc.scalar.activation(out=mv[:, 1:2], in_=mv[:, 1:2], func=AF.Sqrt,
                                     bias=eps_sb, scale=1.0)
                nc.vector.reciprocal(out=mv[:, 1:2], in_=mv[:, 1:2])
                xo = work.tile([128, D], f32, tag="xo", name="xo")
                nc.vector.tensor_scalar(out=xo, in0=x,
                                        scalar1=mv[:, 0:1], scalar2=mv[:, 1:2],
                                        op0=Alu.subtract, op1=Alu.mult)
                nc.vector.tensor_mul(out=xo, in0=xo, in1=gamma_sb)
                nc.vector.tensor_add(out=xo, in0=xo, in1=beta_sb)
                nc.sync.dma_start(out=y[t * 128:(t + 1) * 128, :], in_=xo)

            # ---- emission schedule ------------------------------------------
            for st in range(ST):
                k_proj(0, st)
            q_proj(0, 0)

            # attention(0,0): v_proj runs two groups ahead of ctx; k_proj(1)
            # spread over the block.
            fill00 = [lambda st=st: k_proj(1, st) for st in range(ST)]

            def extra00(g):
                for sc in (4 + 2 * g, 5 + 2 * g):
                    if sc < SC:
                        v_proj(sc)
                if g % 2 == 1 and fill00:
                    fill00.pop(0)()

            for sc in range(4):
                v_proj(sc)
            attention(0, 0, extra=extra00)

            def mk_extra(units, per_group, start_g=0):
                def extra(g):
                    if g < start_g:
                        return
                    for _ in range(per_group):
                        if units:
                            units.pop(0)()
                return extra

            q_proj(1, 0)
            rest00 = [lambda st=st: q_proj(0, st) for st in range(1, ST)]
            attention(1, 0, extra=mk_extra(rest00, 2))

            for t in range(1, ST):
                units = fc_rs_units(t - 1)
                units.append(lambda t=t: q_proj(1, t))
                attention(0, t, extra=mk_extra(units, 3, start_g=1))
                for u in units:
                    u()
                units2 = [lambda t=t: post_rs_a(t - 1), lambda t=t: post_rs_b(t - 1)]

                def extra2(g, u2=units2):
                    if g == 3 or g == 5:
                        if u2:
                            u2.pop(0)()
                attention(1, t, extra=extra2)
                for u in units2:
                    u()
            qkv_ctx.close()

            # tail: fc + split RS for the last slab, then its post-processing
            for u in fc_rs_units(ST - 1):
                u()
            post_rs_a(ST - 1)
            post_rs_b(ST - 1)

    nc.compile()
    return nc


def kernel(q, k, v, w_qs, w_ks, w_vs, w_fc, ln_gamma, ln_beta):
    from concourse import bass_utils

    if "nc" not in _CACHE:
        _CACHE["nc"] = _build()
    nc = _CACHE["nc"]

    f8 = ml_dtypes.float8_e4m3
    q = np.asarray(q, np.float32)
    k = np.asarray(k, np.float32)
    v = np.asarray(v, np.float32)
    w_fc = np.asarray(w_fc, np.float32)

    in_maps = []
    for i in range(N_CORES):
        bi, hg = i // 4, i % 4
        cs = slice(hg * CSL, (hg + 1) * CSL)
        # rows this core ends up with: per slab t, the split ReduceScatter
        # leaves it rows [t*512 + 64*hg, +64) and [t*512 + 256 + 64*hg, +64).
        row_idx = np.concatenate(
            [np.arange(t * 512 + half * 256 + hg * 64,
                       t * 512 + half * 256 + (hg + 1) * 64)
             for t in range(4) for half in range(2)])
        in_maps.append({
            "qT": np.ascontiguousarray(q[bi].T).astype(f8),
            "kT": np.ascontiguousarray(k[bi].T).astype(f8),
            "vT": np.ascontiguousarray(v[bi].T).astype(f8),
            "wq": np.ascontiguousarray(np.asarray(w_qs, np.float32)[:, cs]).astype(f8),
            "wk": np.ascontiguousarray(np.asarray(w_ks, np.float32)[:, cs]).astype(f8),
            "wv": np.ascontiguousarray(np.asarray(w_vs, np.float32)[:, cs] * 16.0).astype(f8),
            "wfc": np.ascontiguousarray(w_fc[cs, :] * 16.0).astype(f8),
            "qres": np.ascontiguousarray(q[bi][row_idx]),
            "gamma": np.ascontiguousarray(np.asarray(ln_gamma, np.float32)),
            "beta": np.ascontiguousarray(np.asarray(ln_beta, np.float32)),
        })

    run_kwargs = dict(_CACHE.get("run_kwargs", {}))
    res = bass_utils.run_bass_kernel_spmd(nc, in_maps, core_ids=list(range(N_CORES)),
                                          **run_kwargs)
    _CACHE["last_res"] = res
    out = np.empty((B, N, D), np.float32)
    for i in range(N_CORES):
        bi, hg = i // 4, i % 4
        yi = res.results[i]["y"]
        for t in range(4):
            for half in range(2):
                rows = slice(t * 512 + half * 256 + hg * 64,
                             t * 512 + half * 256 + (hg + 1) * 64)
                out[bi, rows, :] = yi[t * 128 + half * 64:t * 128 + (half + 1) * 64, :]
    return out


# revision 16
# speedup vs baseline: 1.1694x; 1.1694x over previous
"""Multi-head attention (b=2, n=2048, d_model=1024, h=16, d_k=d_v=64) + relu(fc) +
residual + LayerNorm, sharded over 8 NeuronCores.

Sharding: core i = (batch bi = i//4) x (head-group hg = i%4, 4 heads each).

v2 design (exp-paced pipeline):
- The scalar-engine exp of the 4 heads x 2048 x 2048 scores (~17M elements at
  ~1 elem/lane/cycle) is the hard floor (~140us); every other engine is
  scheduled to stream underneath it.  Tensor-engine execution order ==
  emission order, so the kernel emits, per score group: scores(g) [bf16, two
  heads row-paired], exp(g) [fp8 out], ctx(g-1) [fp8 DoubleRow over the chunk
  pair], plus "filler" matmuls (projections / fc) that are never gated on
  recent results.  This keeps the PE warm (no >3.4us idle, no HAM
  re-throttle) and the scalar engine saturated.
- fp8e4 DoubleRow halves projection/ctx/fc matmul stream time (contraction
  256 per pass).  The attention path contributes only ~1% of the output
  magnitude (residual + LN dominate), so fp8 there is numerically safe.  wv
  and wfc are pre-scaled x16 so fp8 ctx values avoid subnormals; the x1/256
  is folded into the relu's tensor_scalar.
- A ones column rides in the v weights so the softmax denominator lands in
  psum row 64 of the ctx matmul; reciprocal via the fast-approx DVE op and a
  DRAM round-trip broadcast.
- fc partials ReduceScatter (4 ranks) per 512-query slab, split in two
  256-row chunks; relu+residual+LN run per-slab one block after the RS was
  issued so no engine FIFO head-blocks on the collective.
"""

import numpy as np
import ml_dtypes
from contextlib import ExitStack

B = 2
N = 2048
D = 1024
H = 16
DK = 64
HL = H // 4          # heads per core
CSL = HL * DK        # 256 per-core fc contraction
ROWS = N // 4        # 512 output rows per core
VW = 80              # padded ctx weight cols (64 v + 1 ones + 15 pad)
LN_EPS = 1e-6
N_CORES = 8
CTX_FP8 = True       # fp8 DoubleRow ctx path (False: bf16 per-chunk ctx)
RECIP_APPROX = 2     # 0: exact; 2: copy to SBUF then approx (PSUM-in approx is broken)
LN_POW = False       # Alu.pow fails NEFF compile; keep ACT Sqrt + DVE recip

_CACHE = {}


def _build():
    import concourse.bass as bass
    import concourse.tile as tile
    import concourse.mybir as mybir
    from concourse import bacc

    bf16 = mybir.dt.bfloat16
    fp8 = mybir.dt.float8e4
    f32 = mybir.dt.float32
    AF = mybir.ActivationFunctionType
    Alu = mybir.AluOpType
    DR = mybir.MatmulPerfMode.DoubleRow

    nc = bacc.Bacc("TRN2", target_bir_lowering=False, debug=False,
                   num_devices=N_CORES)

    qT = nc.dram_tensor("qT", [D, N], fp8, kind="ExternalInput").ap()
    kT = nc.dram_tensor("kT", [D, N], fp8, kind="ExternalInput").ap()
    vT = nc.dram_tensor("vT", [D, N], fp8, kind="ExternalInput").ap()
    wq = nc.dram_tensor("wq", [D, CSL], fp8, kind="ExternalInput").ap()
    wk = nc.dram_tensor("wk", [D, CSL], fp8, kind="ExternalInput").ap()
    wv = nc.dram_tensor("wv", [D, CSL], fp8, kind="ExternalInput").ap()
    wfc = nc.dram_tensor("wfc", [CSL, D], fp8, kind="ExternalInput").ap()
    qres = nc.dram_tensor("qres", [ROWS, D], f32, kind="ExternalInput").ap()
    gamma = nc.dram_tensor("gamma", [D], f32, kind="ExternalInput").ap()
    beta = nc.dram_tensor("beta", [D], f32, kind="ExternalInput").ap()
    y = nc.dram_tensor("y", [ROWS, D], f32, kind="ExternalOutput").ap()

    KC = D // 128     # 8 contraction chunks for projections
    KP = KC // 2      # 4 DoubleRow chunk pairs
    ST = N // 512     # 4 seq tiles of 512 queries
    SC = N // 128     # 16 seq chunks of 128 keys
    G = 2             # key chunks per group (exp batch == DoubleRow pair)
    NG = SC // G

    with tile.TileContext(nc) as tc:
        with ExitStack() as ctx:
            persist = ctx.enter_context(tc.tile_pool(name="persist", bufs=1))
            work = ctx.enter_context(tc.tile_pool(name="work", bufs=2))
            epool = ctx.enter_context(tc.tile_pool(name="epool", bufs=4))
            pat = ctx.enter_context(tc.tile_pool(name="pat", bufs=1, space="PSUM"))
            dram = ctx.enter_context(tc.tile_pool(name="dram", bufs=2, space="DRAM"))
            qkv_ctx = ExitStack()
            qkv = qkv_ctx.enter_context(tc.tile_pool(name="qkv", bufs=1))

            # PSUM: "s" score tiles [128,2,512] (2 banks) x3 = 6 banks;
            # "c" ctx tiles [80,512] (1 bank) x2.  Projection/fc psums borrow
            # "s" slots.
            def ps_s():
                return pat.tile([128, G, 512], f32, tag="s", name="ps_s", bufs=3)

            def ps_c():
                return pat.tile([VW, 512], f32, tag="c", name="ps_c", bufs=2)

            def ps_f(n=512):
                return pat.tile([128, n], f32, tag="s", name="ps_f", bufs=3)

            # ---- input tiles -------------------------------------------------
            qT_sb = qkv.tile([128, KC, N], fp8, tag="qT", name="qT")
            kT_sb = qkv.tile([128, KC, N], fp8, tag="kT", name="kT")
            vT_sb = qkv.tile([128, KC, N], fp8, tag="vT", name="vT")
            wq_sb = qkv.tile([128, KC, CSL], fp8, tag="wq", name="wq")
            wk_sb = qkv.tile([128, KC, CSL], fp8, tag="wk", name="wk")
            wv_sb = qkv.tile([128, KC, CSL], fp8, tag="wv", name="wv")
            wfc_sb = persist.tile([128, CSL // 128, D], fp8, tag="wfc", name="wfc")
            qres_sb = persist.tile([128, ST, D], f32, tag="qres", name="qres")
            gamma_sb = persist.tile([128, D], f32, tag="gamma", name="gamma")
            beta_sb = persist.tile([128, D], f32, tag="beta", name="beta")
            eps_sb = persist.tile([128, 1], f32, tag="eps", name="eps")

            # DMA issue order == arrival order: weights first, then kT in
            # half-sequence chunks (1KB lines), qT first half, vT, rest.
            def load_half(sb, src, h):
                for kc in range(KC):
                    nc.sync.dma_start(
                        out=sb[:, kc, h * 1024:(h + 1) * 1024],
                        in_=src[kc * 128:(kc + 1) * 128, h * 1024:(h + 1) * 1024])

            nc.sync.dma_start(out=wq_sb, in_=wq.rearrange("(c p) m -> p c m", p=128))
            nc.sync.dma_start(out=wk_sb, in_=wk.rearrange("(c p) m -> p c m", p=128))
            load_half(kT_sb, kT, 0)
            load_half(kT_sb, kT, 1)
            load_half(qT_sb, qT, 0)
            nc.sync.dma_start(out=wv_sb, in_=wv.rearrange("(c p) m -> p c m", p=128))
            load_half(vT_sb, vT, 0)
            load_half(qT_sb, qT, 1)
            load_half(vT_sb, vT, 1)
            nc.sync.dma_start(out=wfc_sb, in_=wfc.rearrange("(c p) n -> p c n", p=128))
            nc.sync.dma_start(out=qres_sb, in_=qres.rearrange("(c p) n -> p c n", p=128))
            nc.sync.dma_start(out=gamma_sb,
                              in_=bass.AP(tensor=gamma.tensor, offset=gamma.offset,
                                          ap=[[0, 128]] + gamma.ap))
            nc.sync.dma_start(out=beta_sb,
                              in_=bass.AP(tensor=beta.tensor, offset=beta.offset,
                                          ap=[[0, 128]] + beta.ap))
            nc.vector.memset(eps_sb, LN_EPS)

            # ---- persistent activation tiles --------------------------------
            qhT = [persist.tile([128, N], bf16, tag=f"qhT{p}", name=f"qhT{p}") for p in range(2)]
            khT = [persist.tile([128, N], bf16, tag=f"khT{p}", name=f"khT{p}") for p in range(2)]
            # vh[g]: fp8 DoubleRow ctx weights, [keys 128, pair 2, head 4, VW]
            # cols 0-63 = 16*v, col 64 = ones (denominator), 65-79 zero pad.
            vh = [persist.tile([128, G, HL, VW], fp8 if CTX_FP8 else bf16,
                               tag=f"vh{g}", name=f"vh{g}")
                  for g in range(NG)]
            # normalized ctx (x16), fp8, [c 128 (2 heads), cc 2, q N]
            ctxn = persist.tile([128, 2, N], fp8, tag="ctxn", name="ctxn")
            xacc = qres_sb  # relu+residual accumulates in place over the residual

            for g in range(NG):
                nc.vector.memset(vh[g][:, :, :, DK:], 0.0)
                nc.vector.memset(vh[g][:, :, :, DK:DK + 1], 1.0)

            # ---- PE warm-up: dummy matmuls during the initial DMA -----------
            warm = persist.tile([128, 384], bf16, tag="warm", name="warm")
            nc.vector.memset(warm, 0.0)
            for i in range(16):
                ps = ps_f(256)
                nc.tensor.matmul(ps, warm[:, 0:128], warm[:, 0:256],
                                 start=True, stop=True)

            # ---- projections (fp8 DoubleRow, contraction pairs over kc) -----
            def proj(dst, p, st, w_sb, src):
                ps = ps_f()
                for kp in range(KP):
                    nc.tensor.matmul(
                        ps,
                        w_sb[:, 2 * kp:2 * kp + 2, p * 128:(p + 1) * 128],
                        src[:, 2 * kp:2 * kp + 2, st * 512:(st + 1) * 512],
                        start=(kp == 0), stop=(kp == KP - 1), perf_mode=DR)
                nc.vector.tensor_copy(out=dst[p][:, st * 512:(st + 1) * 512], in_=ps)

            def k_proj(p, st):
                proj(khT, p, st, wk_sb, kT_sb)

            def q_proj(p, st):
                proj(qhT, p, st, wq_sb, qT_sb)

            def v_proj(sc):
                # out: [seq 128, h*dk 256] = vT_chunk.T @ (16*wv); lands in the
                # DoubleRow weight tile for group sc//2, pair sc%2.
                ps = ps_f(CSL)
                for kp in range(KP):
                    nc.tensor.matmul(
                        ps,
                        vT_sb[:, 2 * kp:2 * kp + 2, sc * 128:(sc + 1) * 128],
                        wv_sb[:, 2 * kp:2 * kp + 2, :],
                        start=(kp == 0), stop=(kp == KP - 1), perf_mode=DR)
                nc.vector.tensor_copy(
                    out=vh[sc // G][:, sc % G, :, 0:DK],
                    in_=ps.rearrange("p (h d) -> p h d", h=HL))

            # ---- attention block (p, t): exp-paced emission -----------------
            def attention(p, t, extra=None):
                pc = [ps_c() for _ in range(2)]
                ppss = {}
                pse = {}
                for g in range(NG):
                    for s in range(2):
                        lo, hi = 64 * s, 64 * (s + 1)
                        ppss[s] = ps_s()
                        for j in range(G):
                            kc = g * G + j
                            nc.tensor.matmul(
                                ppss[s][:, j, :],
                                khT[p][lo:hi, kc * 128:(kc + 1) * 128],
                                qhT[p][lo:hi, t * 512:(t + 1) * 512],
                                start=True, stop=True)
                    for s in range(2):
                        pse[(g, s)] = epool.tile([128, G, 512],
                                                 fp8 if CTX_FP8 else bf16,
                                                 tag="e", name="e")
                        nc.scalar.activation(out=pse[(g, s)], in_=ppss[s], func=AF.Exp,
                                             scale=1.0 / float(np.sqrt(DK)))

                    def ctx_mm(gg, s, stop):
                        if CTX_FP8:
                            nc.tensor.matmul(
                                pc[s], vh[gg][:, :, 2 * p + s, :], pse[(gg, s)],
                                start=(gg == 0), stop=stop, perf_mode=DR)
                        else:
                            for j in range(G):
                                nc.tensor.matmul(
                                    pc[s][0:DK + 1, :],
                                    vh[gg][:, j, 2 * p + s, 0:DK + 1],
                                    pse[(gg, s)][:, j, :],
                                    start=(gg == 0 and j == 0),
                                    stop=(stop and j == G - 1))

                    if g > 0:
                        for s in range(2):
                            ctx_mm(g - 1, s, False)
                    if extra is not None:
                        extra(g)
                for s in range(2):
                    ctx_mm(NG - 1, s, True)
                # normalization: rb = 1/denominator broadcast via DRAM round
                # trip; ctxn = ctx16 * rb (fp8 out).
                rbs = []
                for s in range(2):
                    rb1 = work.tile([1, 512], f32, tag="rb1", name="rb1")
                    if RECIP_APPROX == 2:
                        rb1c = work.tile([1, 512], f32, tag="rb1c", name="rb1c")
                        nc.vector.tensor_copy(out=rb1c, in_=pc[s][DK:DK + 1, :])
                        nc.vector.reciprocal_approx_fast(out=rb1, in_=rb1c)
                    elif RECIP_APPROX == 1:
                        nc.vector.reciprocal_approx_fast(out=rb1, in_=pc[s][DK:DK + 1, :])
                    else:
                        nc.vector.reciprocal(out=rb1, in_=pc[s][DK:DK + 1, :])
                    r_dram = dram.tile([1, 512], f32, tag="rd", name="rd")
                    nc.gpsimd.dma_start(out=r_dram, in_=rb1)
                    rb = work.tile([DK, 512], f32, tag="rb", name="rb")
                    nc.gpsimd.dma_start(
                        out=rb,
                        in_=bass.AP(tensor=r_dram.tensor, offset=r_dram.offset,
                                    ap=[[0, DK]] + r_dram.ap[1:]))
                    rbs.append(rb)
                for s in range(2):
                    cun = work.tile([DK, 512], f32, tag="cun", name="cun")
                    nc.vector.tensor_copy(out=cun, in_=pc[s][0:DK, :])
                    nc.vector.tensor_mul(
                        out=ctxn[64 * s:64 * (s + 1), p, t * 512:(t + 1) * 512],
                        in0=cun, in1=rbs[s])

            # ---- fc + ReduceScatter per slab --------------------------------
            rs_bufs = {}

            def fc_tile(t, qq, nh):
                rs_in = rs_bufs[t][0]
                qc = t * 4 + qq
                ps = ps_f()
                nc.tensor.matmul(
                    ps,
                    ctxn[:, :, qc * 128:(qc + 1) * 128],
                    wfc_sb[:, :, nh * 512:(nh + 1) * 512],
                    start=True, stop=True, perf_mode=DR)
                fcs = work.tile([128, 512], bf16, tag="fcs", name="fcs")
                nc.vector.tensor_copy(out=fcs, in_=ps)
                nc.sync.dma_start(
                    out=rs_in[qq * 128:(qq + 1) * 128, nh * 512:(nh + 1) * 512],
                    in_=fcs)

            def rs_issue(t, half):
                # ReduceScatter rows [t*512 + half*256, +256) over 4 ranks;
                # each rank keeps 64 contiguous rows.
                rs_in = rs_bufs[t][0]
                rs_out = dram.tile([64, D], bf16, tag=f"rs_out{half}", name="rs_out")
                rs_bufs[t][1].append(rs_out)
                nc.gpsimd.collective_compute(
                    "ReduceScatter",
                    mybir.AluOpType.add,
                    replica_groups=[[0, 1, 2, 3], [4, 5, 6, 7]],
                    ins=[rs_in[half * 256:(half + 1) * 256, :].opt()],
                    outs=[rs_out.opt()])

            def fc_rs_units(t):
                rs_in = dram.tile([512, D], bf16, tag="rs_in", name="rs_in")
                rs_bufs[t] = (rs_in, [])
                units = []
                for qq in range(4):
                    for nh in range(2):
                        units.append(lambda t=t, qq=qq, nh=nh: fc_tile(t, qq, nh))
                    if qq == 1:
                        units.append(lambda t=t: rs_issue(t, 0))
                    if qq == 3:
                        units.append(lambda t=t: rs_issue(t, 1))
                return units

            def post_rs_a(t):
                # gather the RS result, relu(sum/256) + residual, LN stats
                rs_sb = work.tile([128, D], bf16, tag="rs_sb", name="rs_sb")
                for half, rs_out in enumerate(rs_bufs[t][1]):
                    nc.sync.dma_start(out=rs_sb[64 * half:64 * half + 64, :],
                                      in_=rs_out)
                xr = work.tile([128, D], f32, tag="xr", name="xr")
                nc.vector.tensor_scalar(out=xr, in0=rs_sb,
                                        scalar1=1.0 / 256.0, scalar2=0.0,
                                        op0=Alu.mult, op1=Alu.max)
                nc.vector.tensor_add(out=xacc[:, t, :], in0=xr,
                                     in1=qres_sb[:, t, :])
                x = xacc[:, t, :]
                stats = work.tile([128, 2, 6], f32, tag="stats", name="stats")
                nc.vector.bn_stats(out=stats[:, 0, :], in_=x[:, 0:512])
                nc.vector.bn_stats(out=stats[:, 1, :], in_=x[:, 512:1024])
                mv = work.tile([128, 2], f32, tag="mv", name="mv")
                nc.vector.bn_aggr(out=mv, in_=stats)
                rs_bufs[t] = (None, rs_bufs[t][1], mv)

            def post_rs_b(t):
                mv = rs_bufs[t][2]
                x = xacc[:, t, :]
                if LN_POW:
                    nc.vector.tensor_scalar(out=mv[:, 1:2], in0=mv[:, 1:2],
                                            scalar1=LN_EPS, scalar2=-0.5,
                                            op0=Alu.add, op1=Alu.pow)
                else:
                    nc.scalar.activation(out=mv[:, 1:2], in_=mv[:, 1:2], func=AF.Sqrt,
                                         bias=eps_sb, scale=1.0)
                    nc.vector.reciprocal(out=mv[:, 1:2], in_=mv[:, 1:2])
                xo = work.tile([128, D], f32, tag="xo", name="xo")
                nc.vector.tensor_scalar(out=xo, in0=x,
                                        scalar1=mv[:, 0:1], scalar2=mv[:, 1:2],
                                        op0=Alu.subtract, op1=Alu.mult)
                nc.vector.tensor_mul(out=xo, in0=xo, in1=gamma_sb)
                nc.vector.tensor_add(out=xo, in0=xo, in1=beta_sb)
                nc.sync.dma_start(out=y[t * 128:(t + 1) * 128, :], in_=xo)

            # ---- emission schedule ------------------------------------------
            for st in range(ST):
                k_proj(0, st)
            q_proj(0, 0)

            # attention(0,0): v_proj runs two groups ahead of ctx; k_proj(1)
            # spread over the block.
            fill00 = [lambda st=st: k_proj(1, st) for st in range(ST)]

            def extra00(g):
                for sc in (4 + 2 * g, 5 + 2 * g):
                    if sc < SC:
                        v_proj(sc)
                if g % 2 == 1 and fill00:
                    fill00.pop(0)()

            for sc in range(4):
                v_proj(sc)
            attention(0, 0, extra=extra00)

            def mk_extra(units, per_group, start_g=0):
                def extra(g):
                    if g < start_g:
                        return
                    for _ in range(per_group):
                        if units:
                            units.pop(0)()
                return extra

            q_proj(1, 0)
            rest00 = [lambda st=st: q_proj(0, st) for st in range(1, ST)]
            attention(1, 0, extra=mk_extra(rest00, 2))

            # slab t's fc+RS is issued in block (0,t+1); its post-processing
            # (which waits on the collective) runs two block-pairs later so no
            # engine FIFO ever head-blocks on an in-flight ReduceScatter.
            for t in range(1, ST):
                units = fc_rs_units(t - 1)
                units.append(lambda t=t: q_proj(1, t))
                if t >= 2:
                    units.append(lambda t=t: post_rs_a(t - 2))
                    units.append(lambda t=t: post_rs_b(t - 2))
                attention(0, t, extra=mk_extra(units, 3, start_g=1))
                for u in units:
                    u()
                attention(1, t)
            qkv_ctx.close()

            # tail: fc + split RS for the last slab; slab 2's post fills the
            # gap while the last collective flies.
            units = fc_rs_units(ST - 1)
            for i, u in enumerate(units):
                u()
                if i == 5:
                    post_rs_a(ST - 2)
            post_rs_b(ST - 2)
            post_rs_a(ST - 1)
            post_rs_b(ST - 1)

    nc.compile()
    return nc


def kernel(q, k, v, w_qs, w_ks, w_vs, w_fc, ln_gamma, ln_beta):
    from concourse import bass_utils

    if "nc" not in _CACHE:
        _CACHE["nc"] = _build()
    nc = _CACHE["nc"]

    f8 = ml_dtypes.float8_e4m3
    q = np.asarray(q, np.float32)
    k = np.asarray(k, np.float32)
    v = np.asarray(v, np.float32)
    w_fc = np.asarray(w_fc, np.float32)

    in_maps = []
    for i in range(N_CORES):
        bi, hg = i // 4, i % 4
        cs = slice(hg * CSL, (hg + 1) * CSL)
        # rows this core ends up with: per slab t, the split ReduceScatter
        # leaves it rows [t*512 + 64*hg, +64) and [t*512 + 256 + 64*hg, +64).
        row_idx = np.concatenate(
            [np.arange(t * 512 + half * 256 + hg * 64,
                       t * 512 + half * 256 + (hg + 1) * 64)
             for t in range(4) for half in range(2)])
        in_maps.append({
            "qT": np.ascontiguousarray(q[bi].T).astype(f8),
            "kT": np.ascontiguousarray(k[bi].T).astype(f8),
            "vT": np.ascontiguousarray(v[bi].T).astype(f8),
            "wq": np.ascontiguousarray(np.asarray(w_qs, np.float32)[:, cs]).astype(f8),
            "wk": np.ascontiguousarray(np.asarray(w_ks, np.float32)[:, cs]).astype(f8),
            "wv": np.ascontiguousarray(np.asarray(w_vs, np.float32)[:, cs] * 16.0).astype(f8),
            "wfc": np.ascontiguousarray(w_fc[cs, :] * 16.0).astype(f8),
            "qres": np.ascontiguousarray(q[bi][row_idx]),
            "gamma": np.ascontiguousarray(np.asarray(ln_gamma, np.float32)),
            "beta": np.ascontiguousarray(np.asarray(ln_beta, np.float32)),
        })

    run_kwargs = dict(_CACHE.get("run_kwargs", {}))
    res = bass_utils.run_bass_kernel_spmd(nc, in_maps, core_ids=list(range(N_CORES)),
                                          **run_kwargs)
    _CACHE["last_res"] = res
    out = np.empty((B, N, D), np.float32)
    for i in range(N_CORES):
        bi, hg = i // 4, i % 4
        yi = res.results[i]["y"]
        for t in range(4):
            for half in range(2):
                rows = slice(t * 512 + half * 256 + hg * 64,
                             t * 512 + half * 256 + (hg + 1) * 64)
                out[bi, rows, :] = yi[t * 128 + half * 64:t * 128 + (half + 1) * 64, :]
    return out
